# revision 1
# baseline (speedup 1.0000x reference)
"""CFConv fused GNN message-passing kernel for 8 Trainium2 NeuronCores.

Strategy (edge-parallel, dst-sharded):
- Host sorts edges by dst and buckets them to 8 cores by dst range (12500
  nodes/core). Within a core, edges are grouped by 128-node dst-groups and
  padded to 128-edge chunks. As part of sharding, the host gathers x[src]
  into the per-edge layout and stacks it with the radial basis (both
  transposed + bias/ones rows) into one combined [116, S] operand.
- Device, per 128-edge chunk: ONE matmul with a block-diagonal weight
  [[Wpre_aug, 0], [0, Wrad_aug]] computes hv|filt [128e, 128] into PSUM;
  ACT copies the hv half to SBUF; DVE multiplies hv*filt into msg; DVE
  builds onehot(dstrel) vs iota; a second matmul accumulates
  msg.T @ onehot into the group's PSUM hT [64, 128n] (segment sum).
- Per group: hT + ones row -> matmul with Wpost_aug (bias folded) -> SiLU
  on ACT -> DMA out transposed. Host concats + transposes the output.
"""
import sys
sys.path.insert(0, "/opt/trn_rl_repo")
import os
import numpy as np

N_NODES = 100000
N_EDGES = 1600000
D_IN = 64
D_RAD = 50
D_H = 64
D_OUT = 64
N_CORES = 8
NPC = N_NODES // N_CORES          # nodes per core
P = 128
NG = (NPC + P - 1) // P           # dst groups per core (98)
COMB = D_IN + 1 + D_RAD + 1       # 116 combined rows

_EXEC = {}


def _prep(x, edge_basis, src, dst, dt_np):
    """Host-side sharding: per-core input dicts + program shape info."""
    order = np.argsort(dst, kind="stable")
    dst_s = dst[order].astype(np.int64)
    src_s = src[order].astype(np.int64)

    core_bounds = np.searchsorted(dst_s, np.arange(N_CORES + 1) * NPC)
    counts = np.zeros((N_CORES, NG), dtype=np.int64)
    core_edges = []
    for c in range(N_CORES):
        lo, hi = core_bounds[c], core_bounds[c + 1]
        e = order[lo:hi]
        d_rel = dst_s[lo:hi] - c * NPC
        g = d_rel // P
        counts[c] = np.bincount(g, minlength=NG)
        core_edges.append((e, d_rel, src_s[lo:hi], g))

    chunks_g = np.maximum(1, (counts.max(axis=0) + P - 1) // P)
    offs = np.concatenate([[0], np.cumsum(chunks_g)])
    SC = int(offs[-1])
    S = SC * P

    in_maps = []
    for c in range(N_CORES):
        e, d_rel, s_ids, g = core_edges[c]
        gstart = np.concatenate([[0], np.cumsum(counts[c])])
        rank = np.arange(len(e)) - gstart[g]
        slot = (offs[g] * P + rank).astype(np.int64)

        comb = np.zeros((S, COMB), dtype=np.float32)
        comb[slot, :D_IN] = x[s_ids]
        comb[:, D_IN] = 1.0
        comb[slot, D_IN + 1 : D_IN + 1 + D_RAD] = edge_basis[e]
        comb[:, D_IN + 1 + D_RAD] = 1.0
        combT = np.ascontiguousarray(comb.T).astype(dt_np)

        ohm = np.zeros((P, S), dtype=dt_np)
        ohm[slot % P, (slot // P) * P + (d_rel - g * P)] = 1.0
        in_maps.append({"combT": combT, "ohm": ohm})
    return in_maps, chunks_g, offs, SC, S


def _build(chunks_g, offs, SC, S, dt, ng_limit=None, repeat=1, loop_repeat=None):
    from concourse import bass, bacc, mybir, tile
    f32 = mybir.dt.float32
    ng = NG if ng_limit is None else ng_limit
    BATCH = int(os.environ.get("CFCONV_BATCH", "8"))

    nc = bacc.Bacc(None, target_bir_lowering=False)
    combT = nc.dram_tensor("combT", [COMB, S], dt, kind="ExternalInput")
    ohm = nc.dram_tensor("ohm", [P, S], dt, kind="ExternalInput")
    wblk = nc.dram_tensor("wblk", [COMB, P], dt, kind="ExternalInput")
    wpost = nc.dram_tensor("wpost", [D_H + 1, D_OUT], dt, kind="ExternalInput")
    outT = nc.dram_tensor("outT", [D_OUT, NPC], f32, kind="ExternalOutput")

    with tile.TileContext(nc) as tc:
        with (
            tc.tile_pool(name="const", bufs=1) as const,
            tc.tile_pool(name="io", bufs=int(os.environ.get("CFCONV_IOBUFS", "6"))) as io,
            tc.tile_pool(name="work", bufs=int(os.environ.get("CFCONV_WORKBUFS", "4"))) as work,
            tc.tile_pool(name="ep", bufs=2) as ep,
            tc.tile_pool(name="ps_pair", bufs=int(os.environ.get("CFCONV_PPBUFS", "2")), space="PSUM") as ps_pair,
            tc.tile_pool(name="ps_h", bufs=2, space="PSUM") as ps_h,
            tc.tile_pool(name="ps_ep", bufs=2, space="PSUM") as ps_ep,
        ):
            wblk_t = const.tile([COMB, P], dt, name="wblk_t")
            nc.sync.dma_start(wblk_t[:], wblk[:])
            wpost_t = const.tile([D_H + 1, D_OUT], dt, name="wpost_t")
            nc.sync.dma_start(wpost_t[:], wpost[:])

            import contextlib
            loop_cm = tc.For_i(0, loop_repeat, 1) if loop_repeat else contextlib.nullcontext()
            def emit_ep(pg, pn, pht):
                # epilogue: outT[:, gP:gP+n] = silu(Wpost_aug.T @ [hT; 1])
                haug = ep.tile([D_H + 1, P], dt, tag="haug", name="haug")
                nc.scalar.copy(haug[:D_H, :], pht[:])
                nc.gpsimd.memset(haug[D_H : D_H + 1, :], 1.0)
                o_ps = ps_ep.tile([D_OUT, P], f32, tag="o_ps", name="o_ps")
                nc.tensor.matmul(o_ps[:], lhsT=wpost_t[:], rhs=haug[:], start=True, stop=True)
                o_sb = ep.tile([D_OUT, P], f32, tag="o_sb", name="o_sb")
                nc.scalar.activation(o_sb[:], o_ps[:], mybir.ActivationFunctionType.Silu)
                nc.sync.dma_start(outT[:, pg * P : pg * P + pn], o_sb[:, :pn])

            pending_ep = None
            with loop_cm:
             for _rep in range(repeat):
              for g in range(ng):
                  n_in_g = min(P, NPC - g * P)
                  nch = int(chunks_g[g])
                  j0 = int(offs[g])
                  ht_ps = ps_h.tile([D_H, P], f32, tag="ht_ps", name="ht_ps")
                  kglob = 0
                  pending = None   # (msg, oh, B) awaiting scatter
                  batches = list(range(0, nch, BATCH))
                  for b0 in batches:
                      B = min(BATCH, nch - b0)
                      cs = (j0 + b0) * P
                      cb_t = io.tile([COMB, BATCH * P], dt, tag="cb", name="cb_t")
                      nc.sync.dma_start(cb_t[:, : B * P], combT[:, cs : cs + B * P])

                      pp = ps_pair.tile([P, BATCH, P], f32, tag="pp", name="pp")
                      zone = 4  # chunks per 2KB PSUM zone ([128, 4, 128] f32)
                      for k in range(B):
                          nc.tensor.matmul(
                              pp[:, k, :],
                              lhsT=cb_t[:, k * P : (k + 1) * P],
                              rhs=wblk_t[:],
                              start=(k % zone == 0),
                              stop=(k % zone == zone - 1 or k == B - 1),
                          )
                      if pending_ep is not None:
                          peg, pen, peh = pending_ep
                          emit_ep(peg, pen, peh)
                          pending_ep = None
                      if pending is not None:
                          pmsg, poh, pB = pending
                          for k in range(pB):
                              nc.tensor.matmul(
                                  ht_ps[:],
                                  lhsT=pmsg[:, k, :],
                                  rhs=poh[:, k * P : (k + 1) * P],
                                  start=(kglob == 0), stop=(kglob == nch - 1),
                              )
                              kglob += 1
                      hv_sb = work.tile([P, BATCH, D_H], dt, tag="hv_sb", name="hv_sb")
                      msg = work.tile([P, BATCH, D_H], dt, tag="msg", name="msg")
                      for h0 in range(0, B, 4):
                          hb = min(4, B - h0)
                          nc.scalar.copy(hv_sb[:, h0 : h0 + hb, :], pp[:, h0 : h0 + hb, 0:D_H])
                          nc.vector.tensor_tensor(
                              out=msg[:, h0 : h0 + hb, :],
                              in0=hv_sb[:, h0 : h0 + hb, :],
                              in1=pp[:, h0 : h0 + hb, D_H : 2 * D_H],
                              op=mybir.AluOpType.mult,
                          )
                      oh = io.tile([P, BATCH * P], dt, tag="oh", name="oh")
                      nc.sync.dma_start(oh[:, : B * P], ohm[:, cs : cs + B * P])
                      pending = (msg, oh, B)
                  pmsg, poh, pB = pending
                  for k in range(pB):
                      nc.tensor.matmul(
                          ht_ps[:],
                          lhsT=pmsg[:, k, :],
                          rhs=poh[:, k * P : (k + 1) * P],
                          start=(kglob == 0), stop=(kglob == nch - 1),
                      )
                      kglob += 1

                  pending_ep = (g, n_in_g, ht_ps)
            if pending_ep is not None:
                peg, pen, peh = pending_ep
                emit_ep(peg, pen, peh)
                pending_ep = None
    nc.compile()
    return nc


class _Exec:
    """Build-once PJRT executor (shard_map over 8 cores)."""

    def __init__(self, nc, n_cores):
        import jax
        from jax.sharding import Mesh, PartitionSpec, NamedSharding
        from jax.experimental.shard_map import shard_map
        from concourse import mybir, bass2jax
        from concourse.bass2jax import _bass_exec_p, install_neuronx_cc_hook

        install_neuronx_cc_hook()
        self.jax = jax
        self.n_cores = n_cores
        partition_name = nc.partition_id_tensor.name if nc.partition_id_tensor else None
        in_names, out_names, out_avals, self.zero_shapes = [], [], [], []
        for alloc in nc.m.functions[0].allocations:
            if not isinstance(alloc, mybir.MemoryLocationSet):
                continue
            name = alloc.memorylocations[0].name
            if alloc.kind == "ExternalInput":
                if name != partition_name:
                    in_names.append(name)
            elif alloc.kind == "ExternalOutput":
                shape = tuple(alloc.tensor_shape)
                dtype = mybir.dt.np(alloc.dtype)
                out_names.append(name)
                out_avals.append(jax.core.ShapedArray(shape, dtype))
                self.zero_shapes.append((shape, dtype))
        self.in_names, self.out_names, self.out_avals = in_names, out_names, out_avals
        n_params, n_outs = len(in_names), len(out_avals)
        all_in = in_names + out_names + ([partition_name] if partition_name else [])

        def _body(*args):
            operands = list(args)
            if partition_name is not None:
                operands.append(bass2jax.partition_id_tensor())
            return tuple(_bass_exec_p.bind(
                *operands,
                out_avals=tuple(out_avals),
                in_names=tuple(all_in),
                out_names=tuple(out_names),
                lowering_input_output_aliases=(),
                sim_require_finite=True,
                sim_require_nnan=True,
                nc=nc,
            ))

        devices = jax.devices()[:n_cores]
        self.mesh = Mesh(np.asarray(devices), ("core",))
        self.fn = jax.jit(
            shard_map(_body, mesh=self.mesh,
                      in_specs=(PartitionSpec("core"),) * (n_params + n_outs),
                      out_specs=(PartitionSpec("core"),) * n_outs,
                      check_rep=False),
            donate_argnums=tuple(range(n_params, n_params + n_outs)),
            keep_unused=True,
        )
        self.sharding = NamedSharding(self.mesh, PartitionSpec("core"))

    def put_inputs(self, in_maps):
        return [self.jax.device_put(
                    np.concatenate([np.asarray(m[n]) for m in in_maps], axis=0),
                    self.sharding)
                for n in self.in_names]

    def zeros(self):
        return [self.jax.device_put(
                    np.zeros((self.n_cores * s[0], *s[1:]), d), self.sharding)
                for s, d in self.zero_shapes]

    def run(self, dev_inputs):
        outs = self.fn(*dev_inputs, *self.zeros())
        self.jax.block_until_ready(outs)
        mats = [np.asarray(o) for o in outs]
        return [
            {n: mats[i].reshape(self.n_cores, *self.out_avals[i].shape)[c]
             for i, n in enumerate(self.out_names)}
            for c in range(self.n_cores)
        ]


def _get_exec(x, edge_basis, src, dst, W_pre, b_pre, W_rad, b_rad, W_post, b_post,
              dt_name=None, ng_limit=None):
    from concourse import mybir
    dt_name = dt_name or os.environ.get("CFCONV_DT", "float16")
    dt = {"float32": mybir.dt.float32, "float16": mybir.dt.float16}[dt_name]
    dt_np = {"float32": np.float32, "float16": np.float16}[dt_name]

    in_maps, chunks_g, offs, SC, S = _prep(x, edge_basis, src, dst, dt_np)
    key = (dt_name, SC, S, tuple(chunks_g), ng_limit)
    if key not in _EXEC:
        nc = _build(chunks_g, offs, SC, S, dt, ng_limit)
        _EXEC[key] = _Exec(nc, N_CORES)
    ex = _EXEC[key]

    wblk_np = np.zeros((COMB, P), dtype=np.float32)
    wblk_np[: D_IN, 0:D_H] = W_pre.T
    wblk_np[D_IN, 0:D_H] = b_pre
    wblk_np[D_IN + 1 : D_IN + 1 + D_RAD, D_H : D_H + D_H] = W_rad.T
    wblk_np[D_IN + 1 + D_RAD, D_H : D_H + D_H] = b_rad
    wpost_np = np.vstack([W_post.T, b_post[None, :]]).astype(dt_np)
    for m in in_maps:
        m["wblk"] = wblk_np.astype(dt_np)
        m["wpost"] = wpost_np
    return ex, in_maps


def kernel(x, edge_basis, src, dst, W_pre, b_pre, W_rad, b_rad, W_post, b_post):
    x = np.asarray(x, dtype=np.float32)
    edge_basis = np.asarray(edge_basis, dtype=np.float32)
    ex, in_maps = _get_exec(x, edge_basis, np.asarray(src, np.int32),
                            np.asarray(dst, np.int32),
                            np.asarray(W_pre, np.float32), np.asarray(b_pre, np.float32),
                            np.asarray(W_rad, np.float32), np.asarray(b_rad, np.float32),
                            np.asarray(W_post, np.float32), np.asarray(b_post, np.float32))
    di = ex.put_inputs(in_maps)
    res = ex.run(di)
    outT_full = np.concatenate([res[c]["outT"] for c in range(N_CORES)], axis=1)
    return np.ascontiguousarray(outT_full.T)



# revision 6
# speedup vs baseline: 1.9040x; 1.9040x over previous
"""CFConv fused GNN message-passing kernel for 8 Trainium2 NeuronCores.

Strategy (edge-parallel, dst-sharded, constant scatter pattern):
- Host sorts edges by dst and buckets them to 8 cores by dst range (12500
  nodes/core). Within a core, nodes are PERMUTED by degree (pure indexing,
  un-permuted when assembling the output) and packed into 64-node dst
  groups of similar degree. Each chunk of 128 edge slots holds exactly 2
  slots per node (slot p -> node p//2), so the scatter one-hot is one
  CONSTANT [128, 64] matrix for the whole kernel. A group with max degree
  d needs ceil(d/2) chunks; degree sorting keeps padding ~5%.
- Host gathers x[src] and stacks it with a validity-mask row and the
  radial basis into combT [115, S] (features major). The mask row zeroes
  hv and filt at padding slots, and its weight-row carries both biases.
- Device, per chunk: matmul1 (lhsT=comb chunk, rhs=wblk) -> [128e, 128]
  f32 PSUM holding hv|filt; ACT copies the hv half to SBUF fp16; DVE
  multiplies hv*filt -> msg fp16; matmul2 (lhsT=msg, rhs=oh_const)
  accumulates the group's ht [64h, 64n] in PSUM (segment sum).
- Per EP groups: copy ht -> SBUF fp16, one matmul with Wpost^T, SiLU with
  per-partition b_post bias on ACT, DMA out as fp16. Host un-permutes.
"""
import sys
sys.path.insert(0, "/opt/trn_rl_repo")
import os
import numpy as np

N_NODES = 100000
N_EDGES = 1600000
D_IN = 64
D_RAD = 50
D_H = 64
D_OUT = 64
N_CORES = 8
NPC = N_NODES // N_CORES          # nodes per core
P = 128
W = 64                            # nodes per dst group
Q = 2                             # edge slots per node per chunk (P // W)
NG = (NPC + W - 1) // W           # dst groups per core (196)
NPOS = NG * W                     # padded node positions per core (12544)
COMB = D_IN + 1 + D_RAD           # 115 combined rows (x, mask, eb)

_EXEC = {}


def _prep(x, edge_basis, src, dst, dt_np):
    """Host-side sharding: per-core input dicts + program shape info."""
    order = np.argsort(dst, kind="stable")
    dst_s = dst[order].astype(np.int64)
    src_s = src[order].astype(np.int64)

    core_bounds = np.searchsorted(dst_s, np.arange(N_CORES + 1) * NPC)
    core_data = []
    need = np.zeros((N_CORES, NG), dtype=np.int64)
    for c in range(N_CORES):
        lo, hi = core_bounds[c], core_bounds[c + 1]
        d_rel = dst_s[lo:hi] - c * NPC
        deg = np.bincount(d_rel, minlength=NPC)
        pos_of_node = np.empty(NG * W, dtype=np.int64)
        deg_pad = np.concatenate([deg, np.zeros(NG * W - NPC, np.int64)])
        sort_nodes = np.argsort(deg_pad, kind="stable")   # ascending degree
        pos_of_node[sort_nodes] = np.arange(NG * W)
        gmax = deg_pad[sort_nodes].reshape(NG, W).max(axis=1)
        need[c] = np.maximum(1, (gmax + Q - 1) // Q)
        core_data.append((lo, hi, d_rel, deg, pos_of_node))

    chunks_g = need.max(axis=0)
    offs = np.concatenate([[0], np.cumsum(chunks_g)])
    SC = int(offs[-1])
    S = SC * P

    ohc = np.zeros((P, W), dtype=dt_np)
    ohc[np.arange(P), np.arange(P) // Q] = 1.0

    in_maps = []
    for c in range(N_CORES):
        lo, hi, d_rel, deg, pos_of_node = core_data[c]
        e = order[lo:hi]
        node_start = np.concatenate([[0], np.cumsum(deg)])
        rank = np.arange(len(e)) - node_start[d_rel]       # i-th edge of node
        pos = pos_of_node[d_rel]
        g = pos // W
        col = pos % W
        slot = (offs[g] + rank // Q) * P + col * Q + rank % Q

        comb = np.zeros((S, COMB), dtype=np.float32)
        comb[slot, :D_IN] = x[src_s[lo:hi]]
        comb[slot, D_IN] = 1.0
        comb[slot, D_IN + 1:] = edge_basis[e]
        combT = np.ascontiguousarray(comb.T).astype(dt_np)
        in_maps.append({"combT": combT, "ohc": ohc, "pos": pos_of_node[:NPC]})
    return in_maps, chunks_g, offs, SC, S


def _pack_weights(W_pre, b_pre, W_rad, b_rad, W_post, b_post, dt_np):
    wblk = np.zeros((COMB, P), dtype=np.float32)
    wblk[:D_IN, 0:D_H] = W_pre.T
    wblk[D_IN, 0:D_H] = b_pre
    wblk[D_IN, D_H:2 * D_H] = b_rad
    wblk[D_IN + 1:, D_H:2 * D_H] = W_rad.T
    wpost = np.ascontiguousarray(W_post.T).astype(dt_np)   # [h, o]
    bpost = np.asarray(b_post, np.float32).reshape(D_OUT, 1)
    return {"wblk": wblk.astype(dt_np), "wpost": wpost, "bpost": bpost}


def _build(chunks_g, offs, SC, S, dt, ng_limit=None, repeat=1, loop_repeat=None):
    from concourse import bass, bacc, mybir, tile
    f32 = mybir.dt.float32
    ng = NG if ng_limit is None else ng_limit
    B = int(os.environ.get("CFCONV_BATCH", "8"))
    TMAX = B * int(os.environ.get("CFCONV_TILEB", "6"))   # chunks per DMA tile
    EP = int(os.environ.get("CFCONV_EP", "4"))            # groups per epilogue
    NA = int(os.environ.get("CFCONV_NA", "99"))           # chunks/batch ACT copies
    HTD = int(os.environ.get("CFCONV_HTD", "1"))          # alternate htcopy engine
    HTB = int(os.environ.get("CFCONV_HTBUFS", "2"))
    IOB = int(os.environ.get("CFCONV_IOBUFS", "3"))
    WB = int(os.environ.get("CFCONV_WORKBUFS", "3"))

    # chunk -> group map
    cg = []
    for g in range(ng):
        cg += [g] * int(chunks_g[g])
    NCH = len(cg)
    last_of_group = {}
    first_of_group = {}
    for i, g in enumerate(cg):
        last_of_group[g] = i
        if g not in first_of_group:
            first_of_group[g] = i

    nc = bacc.Bacc(None, target_bir_lowering=False)
    combT = nc.dram_tensor("combT", [COMB, S], dt, kind="ExternalInput")
    ohc = nc.dram_tensor("ohc", [P, W], dt, kind="ExternalInput")
    wblk = nc.dram_tensor("wblk", [COMB, P], dt, kind="ExternalInput")
    wpost = nc.dram_tensor("wpost", [D_H, D_OUT], dt, kind="ExternalInput")
    bpost = nc.dram_tensor("bpost", [D_OUT, 1], f32, kind="ExternalInput")
    outT = nc.dram_tensor("outT", [D_OUT, NPOS], dt, kind="ExternalOutput")

    with tile.TileContext(nc) as tc:
        with (
            tc.tile_pool(name="const", bufs=1) as const,
            tc.tile_pool(name="io", bufs=IOB) as io,
            tc.tile_pool(name="work", bufs=WB) as work,
            tc.tile_pool(name="stage", bufs=2) as stage,
            tc.tile_pool(name="ps_pair", bufs=2, space="PSUM") as ps_pair,
            tc.tile_pool(name="ps_h", bufs=HTB, space="PSUM") as ps_h,
            tc.tile_pool(name="ps_ep", bufs=2, space="PSUM") as ps_ep,
        ):
            wblk_t = const.tile([COMB, P], dt, name="wblk_t")
            nc.sync.dma_start(wblk_t[:], wblk[:])
            wpost_t = const.tile([D_H, D_OUT], dt, name="wpost_t")
            nc.sync.dma_start(wpost_t[:], wpost[:])
            bpost_t = const.tile([D_OUT, 1], f32, name="bpost_t")
            nc.sync.dma_start(bpost_t[:], bpost[:])
            ohc_t = const.tile([P, W], dt, name="ohc_t")
            nc.sync.dma_start(ohc_t[:], ohc[:])

            import contextlib
            loop_cm = tc.For_i(0, loop_repeat, 1) if loop_repeat else contextlib.nullcontext()

            with loop_cm:
             for _rep in range(repeat):
              ht_tiles = {}          # g -> (psum tile, kcount)
              htst = None            # staging tile for EP groups
              ep_base = 0            # first group of current EP block
              ep_filled = 0
              pending_scatter = None
              pending_groups = []
              cb_t = None
              tile_start = 0

              def emit_epi(ep_base, nfill):
                  nonlocal htst
                  o_ps = ps_ep.tile([D_OUT, EP * W], f32, tag="o_ps", name="o_ps")
                  o_sb = stage.tile([D_OUT, EP * W], dt, tag="o_sb", name="o_sb")
                  nc.tensor.matmul(o_ps[:, : nfill * W], lhsT=wpost_t[:],
                                   rhs=htst[:, : nfill * W], start=True, stop=True)
                  nc.scalar.activation(o_sb[:, : nfill * W], o_ps[:, : nfill * W],
                                       mybir.ActivationFunctionType.Silu,
                                       bias=bpost_t[:, :])
                  lo = ep_base * W
                  hi = min((ep_base + nfill) * W, NPOS)
                  if hi > lo:
                      nc.sync.dma_start(outT[:, lo:hi], o_sb[:, : hi - lo])

              def emit_group_done(gl):
                  """Pair (gl-1, gl) complete: copy to staging; epilogue when
                  the EP block is full. Called only for odd gl (or final)."""
                  nonlocal htst, ep_base, ep_filled
                  if gl % 2 == 0 and gl != ng - 1:
                      return
                  if htst is None:
                      htst = stage.tile([D_H, EP * W], dt, tag="htst", name="htst")
                  pair = gl // 2
                  ht = ht_tiles.pop(pair)
                  npair = 2 if gl % 2 == 1 else 1
                  g0 = pair * 2
                  slot = g0 - ep_base
                  eng = nc.vector.tensor_copy if (HTD and pair % 2) else nc.scalar.copy
                  eng(htst[:, slot * W:(slot + npair) * W],
                      ht[:, :npair, :])
                  ep_filled += npair
                  if ep_filled == EP:
                      emit_epi(ep_base, EP)
                      htst = None
                      ep_base += EP
                      ep_filled = 0

              c0 = 0
              while c0 < NCH:
                  nb = min(B, NCH - c0)
                  if c0 >= tile_start + TMAX or cb_t is None:
                      tile_start = c0
                      tn = min(TMAX, NCH - tile_start)
                      cb_t = io.tile([COMB, TMAX * P], dt, tag="cb", name="cb_t")
                      nc.sync.dma_start(cb_t[:, : tn * P],
                                        combT[:, tile_start * P:(tile_start + tn) * P])
                  pp = ps_pair.tile([P, B, P], f32, tag="pp", name="pp")
                  base = c0 - tile_start
                  for k in range(nb):
                      nc.tensor.matmul(
                          pp[:, k, :],
                          lhsT=cb_t[:, (base + k) * P:(base + k + 1) * P],
                          rhs=wblk_t[:],
                          start=(k % 4 == 0),
                          stop=(k % 4 == 3 or k == nb - 1),
                      )
                  # previous batch's scatter + completed groups
                  if pending_scatter is not None:
                      pmsg, pc0, pnb = pending_scatter
                      for k in range(pnb):
                          g = cg[pc0 + k]
                          pair = g // 2
                          if pair not in ht_tiles:
                              ht_tiles[pair] = ps_h.tile([D_H, 2, W], f32,
                                                         tag="ht", name="ht")
                          ht = ht_tiles[pair]
                          nc.tensor.matmul(
                              ht[:, g % 2, :], lhsT=pmsg[:, k, :], rhs=ohc_t[:],
                              start=(pc0 + k == first_of_group[g]),
                              stop=(pc0 + k == last_of_group[g]),
                          )
                      for k in range(pnb):
                          g = cg[pc0 + k]
                          if pc0 + k == last_of_group[g]:
                              pending_groups.append(g)
                      for gl in pending_groups:
                          emit_group_done(gl)
                      pending_groups = []
                  # hv copy (ACT for first NA chunks, DVE for the rest)
                  hv_sb = work.tile([P, B, D_H], dt, tag="hv_sb", name="hv_sb")
                  na = min(NA, nb)
                  if na > 0:
                      nc.scalar.copy(hv_sb[:, :na, :], pp[:, :na, 0:D_H])
                  if na < nb:
                      nc.vector.tensor_copy(hv_sb[:, na:nb, :], pp[:, na:nb, 0:D_H])
                  msg = work.tile([P, B, D_H], dt, tag="msg", name="msg")
                  nc.vector.tensor_tensor(
                      out=msg[:, :nb, :], in0=hv_sb[:, :nb, :],
                      in1=pp[:, :nb, D_H:2 * D_H], op=mybir.AluOpType.mult,
                  )
                  pending_scatter = (msg, c0, nb)
                  c0 += nb

              # flush
              pmsg, pc0, pnb = pending_scatter
              for k in range(pnb):
                  g = cg[pc0 + k]
                  pair = g // 2
                  if pair not in ht_tiles:
                      ht_tiles[pair] = ps_h.tile([D_H, 2, W], f32, tag="ht",
                                                 name="ht")
                  ht = ht_tiles[pair]
                  nc.tensor.matmul(
                      ht[:, g % 2, :], lhsT=pmsg[:, k, :], rhs=ohc_t[:],
                      start=(pc0 + k == first_of_group[g]),
                      stop=(pc0 + k == last_of_group[g]),
                  )
              for k in range(pnb):
                  g = cg[pc0 + k]
                  if pc0 + k == last_of_group[g]:
                      pending_groups.append(g)
              for gl in pending_groups:
                  emit_group_done(gl)
              pending_groups = []
              if ep_filled > 0:
                  emit_epi(ep_base, ep_filled)
                  htst = None
                  ep_base += ep_filled
                  ep_filled = 0
    nc.compile()
    return nc


class _Exec:
    """Build-once PJRT executor (shard_map over 8 cores)."""

    def __init__(self, nc, n_cores):
        import jax
        from jax.sharding import Mesh, PartitionSpec, NamedSharding
        from jax.experimental.shard_map import shard_map
        from concourse import mybir, bass2jax
        from concourse.bass2jax import _bass_exec_p, install_neuronx_cc_hook

        install_neuronx_cc_hook()
        self.jax = jax
        self.n_cores = n_cores
        partition_name = nc.partition_id_tensor.name if nc.partition_id_tensor else None
        in_names, out_names, out_avals, self.zero_shapes = [], [], [], []
        for alloc in nc.m.functions[0].allocations:
            if not isinstance(alloc, mybir.MemoryLocationSet):
                continue
            name = alloc.memorylocations[0].name
            if alloc.kind == "ExternalInput":
                if name != partition_name:
                    in_names.append(name)
            elif alloc.kind == "ExternalOutput":
                shape = tuple(alloc.tensor_shape)
                dtype = mybir.dt.np(alloc.dtype)
                out_names.append(name)
                out_avals.append(jax.core.ShapedArray(shape, dtype))
                self.zero_shapes.append((shape, dtype))
        self.in_names, self.out_names, self.out_avals = in_names, out_names, out_avals
        n_params, n_outs = len(in_names), len(out_avals)
        all_in = in_names + out_names + ([partition_name] if partition_name else [])

        def _body(*args):
            operands = list(args)
            if partition_name is not None:
                operands.append(bass2jax.partition_id_tensor())
            return tuple(_bass_exec_p.bind(
                *operands,
                out_avals=tuple(out_avals),
                in_names=tuple(all_in),
                out_names=tuple(out_names),
                lowering_input_output_aliases=(),
                sim_require_finite=True,
                sim_require_nnan=True,
                nc=nc,
            ))

        devices = jax.devices()[:n_cores]
        self.mesh = Mesh(np.asarray(devices), ("core",))
        self.fn = jax.jit(
            shard_map(_body, mesh=self.mesh,
                      in_specs=(PartitionSpec("core"),) * (n_params + n_outs),
                      out_specs=(PartitionSpec("core"),) * n_outs,
                      check_rep=False),
            donate_argnums=tuple(range(n_params, n_params + n_outs)),
            keep_unused=True,
        )
        self.sharding = NamedSharding(self.mesh, PartitionSpec("core"))

    def put_inputs(self, in_maps):
        return [self.jax.device_put(
                    np.concatenate([np.asarray(m[n]) for m in in_maps], axis=0),
                    self.sharding)
                for n in self.in_names]

    def zeros(self):
        return [self.jax.device_put(
                    np.zeros((self.n_cores * s[0], *s[1:]), d), self.sharding)
                for s, d in self.zero_shapes]

    def run(self, dev_inputs):
        outs = self.fn(*dev_inputs, *self.zeros())
        self.jax.block_until_ready(outs)
        mats = [np.asarray(o) for o in outs]
        return [
            {n: mats[i].reshape(self.n_cores, *self.out_avals[i].shape)[c]
             for i, n in enumerate(self.out_names)}
            for c in range(self.n_cores)
        ]


def _get_exec(x, edge_basis, src, dst, W_pre, b_pre, W_rad, b_rad, W_post, b_post,
              dt_name=None, ng_limit=None):
    from concourse import mybir
    dt_name = dt_name or os.environ.get("CFCONV_DT", "float16")
    dt = {"float32": mybir.dt.float32, "float16": mybir.dt.float16}[dt_name]
    dt_np = {"float32": np.float32, "float16": np.float16}[dt_name]

    in_maps, chunks_g, offs, SC, S = _prep(x, edge_basis, src, dst, dt_np)
    key = (dt_name, SC, S, tuple(chunks_g), ng_limit)
    if key not in _EXEC:
        nc = _build(chunks_g, offs, SC, S, dt, ng_limit)
        _EXEC[key] = _Exec(nc, N_CORES)
    ex = _EXEC[key]

    wts = _pack_weights(W_pre, b_pre, W_rad, b_rad, W_post, b_post, dt_np)
    for m in in_maps:
        m.update(wts)
    return ex, in_maps


def kernel(x, edge_basis, src, dst, W_pre, b_pre, W_rad, b_rad, W_post, b_post):
    x = np.asarray(x, dtype=np.float32)
    edge_basis = np.asarray(edge_basis, dtype=np.float32)
    ex, in_maps = _get_exec(x, edge_basis, np.asarray(src, np.int32),
                            np.asarray(dst, np.int32),
                            np.asarray(W_pre, np.float32), np.asarray(b_pre, np.float32),
                            np.asarray(W_rad, np.float32), np.asarray(b_rad, np.float32),
                            np.asarray(W_post, np.float32), np.asarray(b_post, np.float32))
    di = ex.put_inputs(in_maps)
    res = ex.run(di)
    out = np.empty((N_NODES, D_OUT), dtype=np.float32)
    for c in range(N_CORES):
        pos = in_maps[c]["pos"]
        out[c * NPC:(c + 1) * NPC] = res[c]["outT"].T[pos].astype(np.float32)
    return out


# revision 8
# speedup vs baseline: 2.4617x; 1.2929x over previous
"""CFConv fused GNN message-passing kernel for 8 Trainium2 NeuronCores.

Strategy (edge-parallel, dst-sharded, constant scatter pattern):
- Host sorts edges by dst and buckets them to 8 cores by dst range (12500
  nodes/core). Within a core, nodes are PERMUTED by degree (pure indexing,
  un-permuted when assembling the output) and packed into 64-node dst
  groups of similar degree. Each chunk of 128 edge slots holds exactly 2
  slots per node (slot p -> node p//2), so the scatter one-hot is one
  CONSTANT [128, 64] matrix for the whole kernel. A group with max degree
  d needs ceil(d/2) chunks; degree sorting keeps padding ~5%.
- Host gathers x[src] and stacks it with a validity-mask row and the
  radial basis into combT [115, S] (features major). The mask row zeroes
  hv and filt at padding slots, and its weight-row carries both biases.
- Device, per chunk: matmul1 (lhsT=comb chunk, rhs=wblk) -> [128e, 128]
  f32 PSUM holding hv|filt; ACT copies the hv half to SBUF fp16; DVE
  multiplies hv*filt -> msg fp16; matmul2 (lhsT=msg, rhs=oh_const)
  accumulates the group's ht [64h, 64n] in PSUM (segment sum).
- Per EP groups: copy ht -> SBUF fp16, one matmul with Wpost^T, SiLU with
  per-partition b_post bias on ACT, DMA out as fp16. Host un-permutes.
"""
import sys
sys.path.insert(0, "/opt/trn_rl_repo")
import os
import numpy as np

N_NODES = 100000
N_EDGES = 1600000
D_IN = 64
D_RAD = 50
D_H = 64
D_OUT = 64
N_CORES = 8
NPC = N_NODES // N_CORES          # nodes per core
P = 128
W = 64                            # nodes per dst group
Q = 2                             # edge slots per node per chunk (P // W)
NG = (NPC + W - 1) // W           # dst groups per core (196)
NPOS = NG * W                     # padded node positions per core (12544)
COMB = D_IN + 1 + D_RAD           # 115 combined rows (x, mask, eb)

_EXEC = {}


def _prep(x, edge_basis, src, dst, dt_np):
    """Host-side sharding: per-core input dicts + program shape info."""
    order = np.argsort(dst, kind="stable")
    dst_s = dst[order].astype(np.int64)
    src_s = src[order].astype(np.int64)

    core_bounds = np.searchsorted(dst_s, np.arange(N_CORES + 1) * NPC)
    core_data = []
    need = np.zeros((N_CORES, NG), dtype=np.int64)
    for c in range(N_CORES):
        lo, hi = core_bounds[c], core_bounds[c + 1]
        d_rel = dst_s[lo:hi] - c * NPC
        deg = np.bincount(d_rel, minlength=NPC)
        pos_of_node = np.empty(NG * W, dtype=np.int64)
        deg_pad = np.concatenate([deg, np.zeros(NG * W - NPC, np.int64)])
        sort_nodes = np.argsort(deg_pad, kind="stable")   # ascending degree
        pos_of_node[sort_nodes] = np.arange(NG * W)
        gmax = deg_pad[sort_nodes].reshape(NG, W).max(axis=1)
        need[c] = np.maximum(1, (gmax + Q - 1) // Q)
        core_data.append((lo, hi, d_rel, deg, pos_of_node))

    chunks_g = need.max(axis=0)
    offs = np.concatenate([[0], np.cumsum(chunks_g)])
    SC = int(offs[-1])
    S = SC * P

    ohc = np.zeros((P, W), dtype=dt_np)
    ohc[np.arange(P), np.arange(P) // Q] = 1.0

    in_maps = []
    for c in range(N_CORES):
        lo, hi, d_rel, deg, pos_of_node = core_data[c]
        e = order[lo:hi]
        node_start = np.concatenate([[0], np.cumsum(deg)])
        rank = np.arange(len(e)) - node_start[d_rel]       # i-th edge of node
        pos = pos_of_node[d_rel]
        g = pos // W
        col = pos % W
        slot = (offs[g] + rank // Q) * P + col * Q + rank % Q

        comb = np.zeros((S, COMB), dtype=np.float32)
        comb[slot, :D_IN] = x[src_s[lo:hi]]
        comb[slot, D_IN] = 1.0
        comb[slot, D_IN + 1:] = edge_basis[e]
        combT = np.ascontiguousarray(comb.T).astype(dt_np)
        in_maps.append({"combT": combT, "ohc": ohc, "pos": pos_of_node[:NPC]})
    return in_maps, chunks_g, offs, SC, S


def _pack_weights(W_pre, b_pre, W_rad, b_rad, W_post, b_post, dt_np):
    wblk = np.zeros((COMB, P), dtype=np.float32)
    wblk[:D_IN, 0:D_H] = W_pre.T
    wblk[D_IN, 0:D_H] = b_pre
    wblk[D_IN, D_H:2 * D_H] = b_rad
    wblk[D_IN + 1:, D_H:2 * D_H] = W_rad.T
    wpost = np.ascontiguousarray(W_post.T).astype(dt_np)   # [h, o]
    bpost = np.asarray(b_post, np.float32).reshape(D_OUT, 1)
    return {"wblk": wblk.astype(dt_np), "wpost": wpost, "bpost": bpost}


def _build(chunks_g, offs, SC, S, dt, ng_limit=None, repeat=1, loop_repeat=None):
    from concourse import bass, bacc, mybir, tile
    f32 = mybir.dt.float32
    ng = NG if ng_limit is None else ng_limit
    B = int(os.environ.get("CFCONV_BATCH", "8"))
    TMAX = B * int(os.environ.get("CFCONV_TILEB", "6"))   # chunks per DMA tile
    EP = int(os.environ.get("CFCONV_EP", "8"))            # groups per epilogue
    NA = int(os.environ.get("CFCONV_NA", "99"))           # chunks/batch ACT copies
    HTD = int(os.environ.get("CFCONV_HTD", "1"))          # alternate htcopy engine
    HTW = int(os.environ.get("CFCONV_HTW", "8"))          # groups per ht psum tile
    HTB = int(os.environ.get("CFCONV_HTBUFS", "1"))
    PPB = int(os.environ.get("CFCONV_PPBUFS", "3"))
    EPB = int(os.environ.get("CFCONV_EPBUFS", "1"))
    DMAQ = int(os.environ.get("CFCONV_DMAQ", "1"))        # split DMA queues
    IOB = int(os.environ.get("CFCONV_IOBUFS", "3"))
    WB = int(os.environ.get("CFCONV_WORKBUFS", "3"))

    # chunk -> group map
    cg = []
    for g in range(ng):
        cg += [g] * int(chunks_g[g])
    NCH = len(cg)
    last_of_group = {}
    first_of_group = {}
    for i, g in enumerate(cg):
        last_of_group[g] = i
        if g not in first_of_group:
            first_of_group[g] = i

    nc = bacc.Bacc(None, target_bir_lowering=False)
    combT = nc.dram_tensor("combT", [COMB, S], dt, kind="ExternalInput")
    ohc = nc.dram_tensor("ohc", [P, W], dt, kind="ExternalInput")
    wblk = nc.dram_tensor("wblk", [COMB, P], dt, kind="ExternalInput")
    wpost = nc.dram_tensor("wpost", [D_H, D_OUT], dt, kind="ExternalInput")
    bpost = nc.dram_tensor("bpost", [D_OUT, 1], f32, kind="ExternalInput")
    outT = nc.dram_tensor("outT", [D_OUT, NPOS], dt, kind="ExternalOutput")

    with tile.TileContext(nc) as tc:
        with (
            tc.tile_pool(name="const", bufs=1) as const,
            tc.tile_pool(name="io", bufs=IOB) as io,
            tc.tile_pool(name="work", bufs=WB) as work,
            tc.tile_pool(name="stage", bufs=2) as stage,
            tc.tile_pool(name="ps_pair", bufs=PPB, space="PSUM") as ps_pair,
            tc.tile_pool(name="ps_h", bufs=HTB, space="PSUM") as ps_h,
            tc.tile_pool(name="ps_ep", bufs=EPB, space="PSUM") as ps_ep,
        ):
            wblk_t = const.tile([COMB, P], dt, name="wblk_t")
            nc.sync.dma_start(wblk_t[:], wblk[:])
            wpost_t = const.tile([D_H, D_OUT], dt, name="wpost_t")
            nc.sync.dma_start(wpost_t[:], wpost[:])
            bpost_t = const.tile([D_OUT, 1], f32, name="bpost_t")
            nc.sync.dma_start(bpost_t[:], bpost[:])
            ohc_t = const.tile([P, W], dt, name="ohc_t")
            nc.sync.dma_start(ohc_t[:], ohc[:])

            import contextlib
            loop_cm = tc.For_i(0, loop_repeat, 1) if loop_repeat else contextlib.nullcontext()

            with loop_cm:
             for _rep in range(repeat):
              ht_tiles = {}          # g -> (psum tile, kcount)
              htst = None            # staging tile for EP groups
              ep_base = 0            # first group of current EP block
              ep_filled = 0
              pending_scatter = None
              pending_groups = []
              cb_t = None
              tile_start = 0

              def emit_epi(ep_base, nfill):
                  nonlocal htst
                  o_ps = ps_ep.tile([D_OUT, EP * W], f32, tag="o_ps", name="o_ps")
                  o_sb = stage.tile([D_OUT, EP * W], dt, tag="o_sb", name="o_sb")
                  nc.tensor.matmul(o_ps[:, : nfill * W], lhsT=wpost_t[:],
                                   rhs=htst[:, : nfill * W], start=True, stop=True)
                  nc.scalar.activation(o_sb[:, : nfill * W], o_ps[:, : nfill * W],
                                       mybir.ActivationFunctionType.Silu,
                                       bias=bpost_t[:, :])
                  lo = ep_base * W
                  hi = min((ep_base + nfill) * W, NPOS)
                  if hi > lo:
                      out_eng = nc.gpsimd if (DMAQ and (ep_base // EP) % 2 == 0) else nc.sync
                      out_eng.dma_start(outT[:, lo:hi], o_sb[:, : hi - lo])

              def emit_group_done(gl):
                  """Quad of HTW groups complete: copy to staging; epilogue
                  when the EP block is full."""
                  nonlocal htst, ep_base, ep_filled
                  if (gl + 1) % HTW != 0 and gl != ng - 1:
                      return
                  if htst is None:
                      htst = stage.tile([D_H, EP * W], dt, tag="htst", name="htst")
                  quad = gl // HTW
                  ht = ht_tiles.pop(quad)
                  nq = gl % HTW + 1
                  g0 = quad * HTW
                  slot = g0 - ep_base
                  eng = nc.vector.tensor_copy if (HTD and quad % 2) else nc.scalar.copy
                  eng(htst[:, slot * W:(slot + nq) * W],
                      ht[:, :nq, :])
                  ep_filled += nq
                  if ep_filled == EP:
                      emit_epi(ep_base, EP)
                      htst = None
                      ep_base += EP
                      ep_filled = 0

              c0 = 0
              while c0 < NCH:
                  nb = min(B, NCH - c0)
                  if c0 >= tile_start + TMAX or cb_t is None:
                      tile_start = c0
                      tn = min(TMAX, NCH - tile_start)
                      cb_t = io.tile([COMB, TMAX * P], dt, tag="cb", name="cb_t")
                      dma_eng = nc.gpsimd if (DMAQ and (tile_start // TMAX) % 2) else nc.sync
                      dma_eng.dma_start(cb_t[:, : tn * P],
                                        combT[:, tile_start * P:(tile_start + tn) * P])
                  pp = ps_pair.tile([P, B, P], f32, tag="pp", name="pp")
                  base = c0 - tile_start
                  for k in range(nb):
                      nc.tensor.matmul(
                          pp[:, k, :],
                          lhsT=cb_t[:, (base + k) * P:(base + k + 1) * P],
                          rhs=wblk_t[:],
                          start=(k % 4 == 0),
                          stop=(k % 4 == 3 or k == nb - 1),
                      )
                  # previous batch's scatter + completed groups
                  if pending_scatter is not None:
                      pmsg, pc0, pnb = pending_scatter
                      for k in range(pnb):
                          g = cg[pc0 + k]
                          quad = g // HTW
                          if quad not in ht_tiles:
                              ht_tiles[quad] = ps_h.tile([D_H, HTW, W], f32,
                                                         tag="ht", name="ht")
                          ht = ht_tiles[quad]
                          nc.tensor.matmul(
                              ht[:, g % HTW, :], lhsT=pmsg[:, k, :], rhs=ohc_t[:],
                              start=(pc0 + k == first_of_group[g]),
                              stop=(pc0 + k == last_of_group[g]),
                          )
                      for k in range(pnb):
                          g = cg[pc0 + k]
                          if pc0 + k == last_of_group[g]:
                              pending_groups.append(g)
                      for gl in pending_groups:
                          emit_group_done(gl)
                      pending_groups = []
                  # hv copy (ACT for first NA chunks, DVE for the rest)
                  hv_sb = work.tile([P, B, D_H], dt, tag="hv_sb", name="hv_sb")
                  na = min(NA, nb)
                  if na > 0:
                      nc.scalar.copy(hv_sb[:, :na, :], pp[:, :na, 0:D_H])
                  if na < nb:
                      nc.vector.tensor_copy(hv_sb[:, na:nb, :], pp[:, na:nb, 0:D_H])
                  msg = work.tile([P, B, D_H], dt, tag="msg", name="msg")
                  nc.vector.tensor_tensor(
                      out=msg[:, :nb, :], in0=hv_sb[:, :nb, :],
                      in1=pp[:, :nb, D_H:2 * D_H], op=mybir.AluOpType.mult,
                  )
                  pending_scatter = (msg, c0, nb)
                  c0 += nb

              # flush
              pmsg, pc0, pnb = pending_scatter
              for k in range(pnb):
                  g = cg[pc0 + k]
                  quad = g // HTW
                  if quad not in ht_tiles:
                      ht_tiles[quad] = ps_h.tile([D_H, HTW, W], f32, tag="ht",
                                                 name="ht")
                  ht = ht_tiles[quad]
                  nc.tensor.matmul(
                      ht[:, g % HTW, :], lhsT=pmsg[:, k, :], rhs=ohc_t[:],
                      start=(pc0 + k == first_of_group[g]),
                      stop=(pc0 + k == last_of_group[g]),
                  )
              for k in range(pnb):
                  g = cg[pc0 + k]
                  if pc0 + k == last_of_group[g]:
                      pending_groups.append(g)
              for gl in pending_groups:
                  emit_group_done(gl)
              pending_groups = []
              if ep_filled > 0:
                  emit_epi(ep_base, ep_filled)
                  htst = None
                  ep_base += ep_filled
                  ep_filled = 0
    nc.compile()
    return nc


class _Exec:
    """Build-once PJRT executor (shard_map over 8 cores)."""

    def __init__(self, nc, n_cores):
        import jax
        from jax.sharding import Mesh, PartitionSpec, NamedSharding
        from jax.experimental.shard_map import shard_map
        from concourse import mybir, bass2jax
        from concourse.bass2jax import _bass_exec_p, install_neuronx_cc_hook

        install_neuronx_cc_hook()
        self.jax = jax
        self.n_cores = n_cores
        partition_name = nc.partition_id_tensor.name if nc.partition_id_tensor else None
        in_names, out_names, out_avals, self.zero_shapes = [], [], [], []
        for alloc in nc.m.functions[0].allocations:
            if not isinstance(alloc, mybir.MemoryLocationSet):
                continue
            name = alloc.memorylocations[0].name
            if alloc.kind == "ExternalInput":
                if name != partition_name:
                    in_names.append(name)
            elif alloc.kind == "ExternalOutput":
                shape = tuple(alloc.tensor_shape)
                dtype = mybir.dt.np(alloc.dtype)
                out_names.append(name)
                out_avals.append(jax.core.ShapedArray(shape, dtype))
                self.zero_shapes.append((shape, dtype))
        self.in_names, self.out_names, self.out_avals = in_names, out_names, out_avals
        n_params, n_outs = len(in_names), len(out_avals)
        all_in = in_names + out_names + ([partition_name] if partition_name else [])

        def _body(*args):
            operands = list(args)
            if partition_name is not None:
                operands.append(bass2jax.partition_id_tensor())
            return tuple(_bass_exec_p.bind(
                *operands,
                out_avals=tuple(out_avals),
                in_names=tuple(all_in),
                out_names=tuple(out_names),
                lowering_input_output_aliases=(),
                sim_require_finite=True,
                sim_require_nnan=True,
                nc=nc,
            ))

        devices = jax.devices()[:n_cores]
        self.mesh = Mesh(np.asarray(devices), ("core",))
        self.fn = jax.jit(
            shard_map(_body, mesh=self.mesh,
                      in_specs=(PartitionSpec("core"),) * (n_params + n_outs),
                      out_specs=(PartitionSpec("core"),) * n_outs,
                      check_rep=False),
            donate_argnums=tuple(range(n_params, n_params + n_outs)),
            keep_unused=True,
        )
        self.sharding = NamedSharding(self.mesh, PartitionSpec("core"))

    def put_inputs(self, in_maps):
        return [self.jax.device_put(
                    np.concatenate([np.asarray(m[n]) for m in in_maps], axis=0),
                    self.sharding)
                for n in self.in_names]

    def zeros(self):
        return [self.jax.device_put(
                    np.zeros((self.n_cores * s[0], *s[1:]), d), self.sharding)
                for s, d in self.zero_shapes]

    def run(self, dev_inputs):
        outs = self.fn(*dev_inputs, *self.zeros())
        self.jax.block_until_ready(outs)
        mats = [np.asarray(o) for o in outs]
        return [
            {n: mats[i].reshape(self.n_cores, *self.out_avals[i].shape)[c]
             for i, n in enumerate(self.out_names)}
            for c in range(self.n_cores)
        ]


def _get_exec(x, edge_basis, src, dst, W_pre, b_pre, W_rad, b_rad, W_post, b_post,
              dt_name=None, ng_limit=None):
    from concourse import mybir
    dt_name = dt_name or os.environ.get("CFCONV_DT", "float16")
    dt = {"float32": mybir.dt.float32, "float16": mybir.dt.float16}[dt_name]
    dt_np = {"float32": np.float32, "float16": np.float16}[dt_name]

    in_maps, chunks_g, offs, SC, S = _prep(x, edge_basis, src, dst, dt_np)
    key = (dt_name, SC, S, tuple(chunks_g), ng_limit)
    if key not in _EXEC:
        nc = _build(chunks_g, offs, SC, S, dt, ng_limit)
        _EXEC[key] = _Exec(nc, N_CORES)
    ex = _EXEC[key]

    wts = _pack_weights(W_pre, b_pre, W_rad, b_rad, W_post, b_post, dt_np)
    for m in in_maps:
        m.update(wts)
    return ex, in_maps


def kernel(x, edge_basis, src, dst, W_pre, b_pre, W_rad, b_rad, W_post, b_post):
    x = np.asarray(x, dtype=np.float32)
    edge_basis = np.asarray(edge_basis, dtype=np.float32)
    ex, in_maps = _get_exec(x, edge_basis, np.asarray(src, np.int32),
                            np.asarray(dst, np.int32),
                            np.asarray(W_pre, np.float32), np.asarray(b_pre, np.float32),
                            np.asarray(W_rad, np.float32), np.asarray(b_rad, np.float32),
                            np.asarray(W_post, np.float32), np.asarray(b_post, np.float32))
    di = ex.put_inputs(in_maps)
    res = ex.run(di)
    out = np.empty((N_NODES, D_OUT), dtype=np.float32)
    for c in range(N_CORES):
        pos = in_maps[c]["pos"]
        out[c * NPC:(c + 1) * NPC] = res[c]["outT"].T[pos].astype(np.float32)
    return out
